# revision 1
# baseline (speedup 1.0000x reference)
"""Trainium2 Bass kernel for nn_CPDist.

Math: with a = exp(h_last @ W.T + b).reshape(B, H, V, R), the reference
computes p_tilde[b,i,j] = sum_r a[b,0,i,r]*a[b,1,j,r], then
  p_eval[b]     = p_tilde[b, p0, p1]
  norm_const[b] = sum_ij p_tilde[b,i,j]
Both factorize over the rank dim, so the (B,V,V) slab is never needed:
  norm_const[b] = sum_r (sum_i a[b,0,i,r]) * (sum_j a[b,1,j,r])
  p_eval[b]     = sum_r a[b,0,p0,r] * a[b,1,p1,r]
The dominant cost is the (B=8, D=1024) x (D, V*R*H=131072) matmul + exp —
HBM-bound on streaming the 512 MB weight matrix.

Sharding: vocab dim V split across 8 cores (512 vocab rows each, for both
horizon slots). Each core streams its (1024, 16384) transposed weight slab
through the PE array against a stationary h^T and applies exp on the scalar
engine, whose accum_out gives the per-(h,r) vocab-sum partials for free.
The 256 gathered rows needed for p_eval are computed exactly in fp32 by a
tiny side matmul (replicated on every core). Host combines the (8,32)
per-core partials.

Per-core W^T slab column order is (h, r, v) so each 512-column matmul chunk
is exactly one (h, r) pair over all 512 local vocab entries.
"""

import os

import numpy as np

import concourse.bacc as bacc
import concourse.bass as bass
import concourse.mybir as mybir
import concourse.tile as tile
from concourse.bass import ts

B, T, D = 8, 128, 1024
V, R, H = 4096, 16, 2
NCORES = 8
VSH = V // NCORES            # vocab rows per core (512)
CHUNK = VSH                  # columns per matmul group
NCHUNK = H * R               # 32 chunks of 512 columns = 16384 per core
KT = D // 128                # 8 contraction tiles
NG = B * H * R               # 256 gathered columns for p_eval

F32 = mybir.dt.float32

# dtype of the streamed weight slab / stationary h for the big matmul.
#   bfloat16: half the HBM traffic, 1 cyc/row on PE
#   float32r: full fp32 storage, mantissa rounded to 11 bits by PE, 1 cyc/row
#   float32:  exact, but 4 cyc/row on PE
_MM_NAME = os.environ.get("CPDIST_MM_DTYPE", "float8e4")
MM_DTYPE = getattr(mybir.dt, _MM_NAME)
# fp8 operands are pre-scaled into e4m3's sweet spot; the activation's scale
# argument undoes S*S on the logits before exp.
MM_SCALE = 1024.0 if MM_DTYPE == mybir.dt.float8e4 else 1.0

# chunks per weight DMA (1, 2, 4, ...) and buffer count for the weight pool
_CPD_DEFAULT = "4" if MM_DTYPE == mybir.dt.float8e4 else "2"
CPD = int(os.environ.get("CPDIST_CPD", _CPD_DEFAULT))
_MM_ITEMSIZE = np.dtype(mybir.dt.np(MM_DTYPE)).itemsize
# keep the weight pool around ~96KB/partition regardless of dtype
_WBUFS_DEFAULT = max(2, (96 * 1024) // (KT * CHUNK * CPD * _MM_ITEMSIZE))
WBUFS = int(os.environ.get("CPDIST_WBUFS", str(_WBUFS_DEFAULT)))
# alternate weight DMAs between the SP and ACT HWDGE rings
ALT_RING = os.environ.get("CPDIST_ALT_RING", "0") == "1"
FILL_RAMP = os.environ.get("CPDIST_FILL_RAMP", "1") == "1"
PSBUFS = int(os.environ.get("CPDIST_PSBUFS", "4"))
GP_ALT = os.environ.get("CPDIST_GP_ALT", "0") == "1"
# sum the exp over vocab on the (otherwise idle) vector engine instead of
# the activation accumulator — shortens the ACT-paced end-of-stream drain
DVE_SUM = os.environ.get("CPDIST_DVE_SUM", "0") == "1"
# issue the first weight DMA via SWDGE so Q7 descriptor emission overlaps the
# HWDGE preamble at kernel start
GP_FIRST = os.environ.get("CPDIST_GP_FIRST", "0") == "1"
# exp two chunks per ACT instruction from a 2-bank psum tile; vocab sums via
# one strided DVE reduce per pair
PAIR = os.environ.get("CPDIST_PAIR", "0") == "1"
# DoubleRow perf mode for the fp8 main matmuls (2 MACs/cell, K=256/matmul)
DR = os.environ.get("CPDIST_DR", "1" if MM_DTYPE == mybir.dt.float8e4 else "0") == "1"

_cached = {}
_fast = {}
_last_results = None


def _round_fp32r(x):
    u = x.view(np.uint32)
    u = (u + np.uint32(0x7FF) + ((u >> np.uint32(12)) & np.uint32(1))) & np.uint32(
        0xFFFFF000
    )
    return u.view(np.float32)


def _to_mm(x, scale=1.0):
    x = np.ascontiguousarray(x, dtype=np.float32)
    if scale != 1.0:
        x = x * np.float32(scale)
    if MM_DTYPE == mybir.dt.float32r:
        return _round_fp32r(x)
    return x.astype(mybir.dt.np(MM_DTYPE))


# WG16: carry the p_eval gather matrix + its h^T in fp16 (halves its DMA
# bytes; the norm path is unaffected and p_eval error stays ~2e-5)
WG16 = os.environ.get("CPDIST_WG16", "1") == "1"
GDT = mybir.dt.float16 if WG16 else mybir.dt.float32

# pack1 (f32) column layout: [htf tiled | wg tiled (fp32 mode only)] | sel |
# onesf | biasg
P1_HTF = 0
P1_WG = P1_HTF + (0 if WG16 else KT * B)
P1_SEL = P1_WG + (0 if WG16 else KT * NG)
P1_ONES = P1_SEL + (0 if WG16 else NG)
P1_BIASG = P1_ONES + B
P1_COLS = P1_BIASG + NG

# pack3 (fp16, WG16 mode): htf tiled | wg tiled
P3_HTF = 0
P3_WG = P3_HTF + KT * B
P3_COLS = P3_WG + KT * NG
P1_ROWS = 1 if WG16 else 128

# pack2 (mm dtype) column layout: ht tiled | ones | DoubleRow ht (padded to
# 16-col pairs so the dual-fp8 LDWEIGHTS pair stride is 16 bytes)
P2_HT = 0
P2_ONES = P2_HT + KT * B
P2_DR = P2_ONES + B
P2_COLS = P2_DR + (KT // 2) * 32


F32R = mybir.dt.float32r


def _build_nc(mm_dtype, nloop=1, use_bias=True):
    nc = bacc.Bacc("TRN2", target_bir_lowering=False)
    pack1 = nc.dram_tensor("pack1", (P1_ROWS, P1_COLS), F32, kind="ExternalInput")
    if WG16:
        pack3 = nc.dram_tensor("pack3", (128, P3_COLS), GDT, kind="ExternalInput")
        selm = nc.dram_tensor("selm", (B, NG), GDT, kind="ExternalInput")
    pack2 = nc.dram_tensor("pack2", (128, P2_COLS), mm_dtype, kind="ExternalInput")
    # weight slab pre-tiled on host as [p, ch, k, v]: every group DMA is a
    # contiguous per-partition slice (16KB runs)
    wt = nc.dram_tensor("wt", (128, NCHUNK * KT * CHUNK), mm_dtype, kind="ExternalInput")
    bias_m = nc.dram_tensor("bias_m", (1, NCHUNK * CHUNK + B), F32R, kind="ExternalInput")
    sg_out = nc.dram_tensor("sg_out", (B, 2 * NCHUNK), F32, kind="ExternalOutput")

    with tile.TileContext(nc) as tc:
        with (
            tc.tile_pool(name="consts", bufs=1) as consts,
            tc.tile_pool(name="wpool", bufs=WBUFS) as wpool,
            tc.tile_pool(name="pspool", bufs=PSBUFS, space="PSUM") as pspool,
            tc.tile_pool(name="psg_pool", bufs=1, space="PSUM") as psg_pool,
            tc.tile_pool(name="epool", bufs=3) as epool,
            tc.tile_pool(name="opool", bufs=1) as opool,
        ):
            if CPD > 1 and FILL_RAMP:
                head = [int(x) for x in os.environ.get("CPDIST_HEAD", "1,1,2").split(",") if x]
                tail_plan = [int(x) for x in os.environ.get("CPDIST_TAIL", "2,1,1").split(",") if x]
                mid = NCHUNK - sum(head) - sum(tail_plan)
                plan = head + [CPD] * (mid // CPD) + \
                    ([mid % CPD] if mid % CPD else []) + tail_plan
            else:
                plan = [CPD] * (NCHUNK // CPD)
            PRE_ISSUE = min(3, len(plan))
            SIDE_AT = 4

            def issue_group(ch0, cpd_g):
                gw = CHUNK * cpd_g
                w_tile = wpool.tile([128, KT * gw], mm_dtype,
                                    padded_shape=[128, KT * CHUNK * CPD],
                                    name=f"w_tile_{ch0}", tag="w_tile")
                weng = nc.gpsimd if ((GP_ALT and ch0 % 2) or
                                     (GP_FIRST and ch0 == 0)) else nc.sync
                weng.dma_start(
                    out=w_tile[:],
                    in_=wt[:, ch0 * KT * CHUNK:(ch0 + cpd_g) * KT * CHUNK],
                )
                return w_tile

            # first weight group goes ahead of everything: its engine time
            # covers the per-DMA HWDGE setup cadence that otherwise leaves
            # ~1us of idle DMA at the start
            pre = [issue_group(0, plan[0])]
            pre_ch = plan[0]

            # bias + pack2 gate the first main-loop psum group; load them
            # before pack1 (which only the side path needs)
            bias_sb = consts.tile([1, NCHUNK * CHUNK + B], F32R)
            if use_bias:
                nc.sync.dma_start(out=bias_sb[:], in_=bias_m[:])
            ones_r = bias_sb[0:1, NCHUNK * CHUNK:NCHUNK * CHUNK + B]
            p2_sb = consts.tile([128, P2_COLS], mm_dtype)
            nc.sync.dma_start(out=p2_sb[:], in_=pack2[:])
            for gi in range(1, PRE_ISSUE):
                pre.append(issue_group(pre_ch, plan[gi]))
                pre_ch += plan[gi]
            p1_sb = consts.tile([P1_ROWS, P1_COLS], F32)
            if WG16:
                p3_sb = consts.tile([128, P3_COLS], GDT)
                sel_tile = consts.tile([B, NG], GDT)

                def htf_k(k):
                    return p3_sb[:, P3_HTF + k * B:P3_HTF + (k + 1) * B]

                def wg_k(k):
                    return p3_sb[:, P3_WG + k * NG:P3_WG + (k + 1) * NG]
            else:
                def htf_k(k):
                    return p1_sb[:, P1_HTF + k * B:P1_HTF + (k + 1) * B]

                def wg_k(k):
                    return p1_sb[:, P1_WG + k * NG:P1_WG + (k + 1) * NG]

            if WG16:
                sel_sb = sel_tile[:]
            else:
                sel_sb = p1_sb[0:B, P1_SEL:P1_SEL + NG]
            onesf_sb = p1_sb[0:1, P1_ONES:P1_ONES + B]
            biasg_sb = p1_sb[0:1, P1_BIASG:P1_BIASG + NG]

            def ht_k(k):
                return p2_sb[:, P2_HT + k * B:P2_HT + (k + 1) * B]

            ones_sb = p2_sb[0:1, P2_ONES:P2_ONES + B]

            sg_sb = opool.tile([B, 2 * NCHUNK], F32)
            s_sb = sg_sb[:, 0:NCHUNK]
            g_sb = sg_sb[:, NCHUNK:2 * NCHUNK]

            def emit_side_path():
                # exact fp32 gathered factors for p_eval; emitted mid-stream
                # so its pack1 DMA and PE work slot into idle gaps instead of
                # delaying the weight stream / first chunks
                nc.sync.dma_start(out=p1_sb[:], in_=pack1[:])
                if WG16:
                    nc.sync.dma_start(out=p3_sb[:], in_=pack3[:])
                    nc.sync.dma_start(out=sel_tile[:], in_=selm[:])
                psg = psg_pool.tile([B, NG], F32)
                for k in range(KT):
                    nc.tensor.matmul(
                        psg[:],
                        lhsT=htf_k(k),
                        rhs=wg_k(k),
                        start=(k == 0),
                        stop=False,
                    )
                nc.tensor.matmul(
                    psg[:], lhsT=onesf_sb, rhs=biasg_sb, start=False, stop=True
                )
                eg = epool.tile([B, NG], F32, tag="eg")
                nc.scalar.activation(eg[:], psg[:], mybir.ActivationFunctionType.Exp)
                # select own-batch columns, then sum over the b' axis
                mg = epool.tile([B, NG], F32, tag="mg")
                nc.vector.tensor_tensor(
                    out=mg[:], in0=eg[:], in1=sel_sb, op=mybir.AluOpType.mult
                )
                nc.vector.tensor_reduce(
                    out=g_sb,
                    in_=mg[:].rearrange("b (c e) -> b c e", e=B),
                    axis=mybir.AxisListType.X,
                    op=mybir.AluOpType.add,
                )

            # --- main path: stream the weight slab ---
            side_emitted = False
            for rep in range(nloop):
              ch0 = 0
              if rep > 0:
                  pre = []
              for gidx, cpd_g in enumerate(plan):
                  gw = CHUNK * cpd_g
                  if rep == 0 and gidx == SIDE_AT and not side_emitted:
                      emit_side_path()
                      side_emitted = True
                  if rep == 0 and gidx < len(pre):
                      w_tile = pre[gidx]
                  else:
                      w_tile = issue_group(ch0, cpd_g)
                  for j2 in range(0, cpd_g, 2 if PAIR else 1):
                      npair = min(2, cpd_g - j2) if PAIR else 1
                      ps = pspool.tile([B, CHUNK * npair], F32, tag="ps",
                                       padded_shape=[B, CHUNK * (2 if PAIR else 1)])
                      for jj in range(npair):
                          j = j2 + jj
                          ch = ch0 + j
                          half = ps[:, jj * CHUNK:(jj + 1) * CHUNK]
                          if use_bias:
                              # bias matmul first: reads only long-resident
                              # tiles, absorbing the psum-slot-free wait
                              nc.tensor.matmul(
                                  half,
                                  lhsT=ones_r,
                                  rhs=bias_sb[:, ts(ch, CHUNK)],
                                  start=True,
                                  stop=False,
                              )
                          jbase = j * KT * CHUNK
                          if DR:
                              for k2 in range(KT // 2):
                                  nc.tensor.matmul(
                                      half,
                                      lhsT=p2_sb[:, P2_DR + k2 * 32:P2_DR + (k2 + 1) * 32]
                                          .rearrange("p (i m) -> p i m", i=2)[:, :, 0:B],
                                      rhs=w_tile[:, jbase + 2 * k2 * CHUNK:
                                                 jbase + (2 * k2 + 2) * CHUNK]
                                          .rearrange("p (i n) -> p i n", i=2),
                                      start=(not use_bias and k2 == 0),
                                      stop=(k2 == KT // 2 - 1),
                                      perf_mode=mybir.MatmulPerfMode.DoubleRow,
                                  )
                          else:
                              for k in range(KT):
                                  nc.tensor.matmul(
                                      half,
                                      lhsT=ht_k(k),
                                      rhs=w_tile[:, jbase + k * CHUNK:jbase + (k + 1) * CHUNK],
                                      start=(not use_bias and k == 0),
                                      stop=(k == KT - 1),
                                  )
                      e_tile = epool.tile([B, CHUNK * npair], F32,
                                          padded_shape=[B, CHUNK * (2 if PAIR else 1)])
                      if PAIR or DVE_SUM:
                          nc.scalar.activation(
                              e_tile[:],
                              ps[:],
                              mybir.ActivationFunctionType.Exp,
                              scale=1.0 / (MM_SCALE * MM_SCALE),
                          )
                          nc.vector.tensor_reduce(
                              out=sg_sb[:, ch0 + j2:ch0 + j2 + npair],
                              in_=e_tile[:].rearrange("b (c v) -> b c v", v=CHUNK),
                              axis=mybir.AxisListType.X,
                              op=mybir.AluOpType.add,
                          )
                      else:
                          nc.scalar.activation(
                              e_tile[:],
                              ps[:],
                              mybir.ActivationFunctionType.Exp,
                              scale=1.0 / (MM_SCALE * MM_SCALE),
                              accum_out=sg_sb[:, ch0 + j2:ch0 + j2 + 1],
                          )
                  ch0 += cpd_g
            if not side_emitted:
                emit_side_path()

            nc.sync.dma_start(out=sg_out[:], in_=sg_sb[:])
    nc.compile()
    return nc


def _get_nc(nloop=1, use_bias=True):
    key = (str(MM_DTYPE), CPD, WBUFS, ALT_RING, DR, FILL_RAMP, PSBUFS, GP_ALT, DVE_SUM, PAIR, WG16, nloop, use_bias)
    if key not in _cached:
        _cached[key] = _build_nc(MM_DTYPE, nloop, use_bias)
    return _cached[key]


def _tile_k(x):
    # (D, N) -> (128, KT*N) with column blocks per contraction tile
    n = x.shape[1]
    return np.ascontiguousarray(
        x.reshape(KT, 128, n).transpose(1, 0, 2).reshape(128, KT * n)
    )


def _prep_core_inputs(W, bias_vec, points, ht):
    W4 = W.reshape(H, V, R, D)
    b3 = bias_vec.reshape(H, V, R)

    # gathered rows for p_eval: column order (h, r, b)
    rows = np.empty((NG,), np.int64)
    for h in range(H):
        for r in range(R):
            for b in range(B):
                rows[(h * R + r) * B + b] = h * V * R + int(points[b, h]) * R + r
    wg = np.ascontiguousarray(W[rows, :].T)           # (D, NG)

    pack1 = np.zeros((P1_ROWS, P1_COLS), np.float32)
    if not WG16:
        pack1[:, P1_HTF:P1_HTF + KT * B] = _tile_k(ht.astype(np.float32))
        pack1[:, P1_WG:P1_WG + KT * NG] = _tile_k(wg)
        for b in range(B):
            pack1[b, P1_SEL + np.arange(NCHUNK) * B + b] = 1.0
    pack1[0, P1_ONES:P1_ONES + B] = 1.0
    pack1[0, P1_BIASG:P1_BIASG + NG] = bias_vec[rows]

    pack2 = np.zeros((128, P2_COLS), np.float32)
    ht_t = _tile_k(ht.astype(np.float32)) * np.float32(MM_SCALE)  # (128, KT*B)
    pack2[:, P2_HT:P2_HT + KT * B] = ht_t
    pack2[0, P2_ONES:P2_ONES + B] = 1.0
    for k2 in range(KT // 2):
        for i in range(2):
            k = 2 * k2 + i
            pack2[:, P2_DR + k2 * 32 + i * 16:P2_DR + k2 * 32 + i * 16 + B] = \
                ht_t[:, k * B:(k + 1) * B]
    pack2 = _to_mm(pack2)

    common = {"pack1": pack1, "pack2": pack2}
    if WG16:
        np16 = mybir.dt.np(GDT)
        pack3 = np.zeros((128, P3_COLS), np16)
        pack3[:, P3_HTF:P3_HTF + KT * B] = _tile_k(ht.astype(np.float32)).astype(np16)
        pack3[:, P3_WG:P3_WG + KT * NG] = _tile_k(wg).astype(np16)
        selm = np.zeros((B, NG), np16)
        for b in range(B):
            selm[b, np.arange(NCHUNK) * B + b] = 1.0
        common["pack3"] = pack3
        common["selm"] = selm

    in_maps = []
    for c in range(NCORES):
        sl = slice(c * VSH, (c + 1) * VSH)
        # (h, v, r, k, p) -> (p, h, r, k, v): chunk-major per partition so
        # group DMAs are contiguous slices
        s5 = W4[:, sl, :, :].reshape(H, VSH, R, KT, 128)
        slab = np.ascontiguousarray(s5.transpose(4, 0, 2, 3, 1))
        slab = _to_mm(slab.reshape(128, NCHUNK * KT * CHUNK), MM_SCALE)
        bc = np.ascontiguousarray(b3[:, sl, :].transpose(0, 2, 1)).reshape(-1)
        bcr = np.empty((1, NCHUNK * CHUNK + B), np.float32)
        bcr[0, :NCHUNK * CHUNK] = bc * np.float32(MM_SCALE * MM_SCALE)
        bcr[0, NCHUNK * CHUNK:] = 1.0
        bc = _round_fp32r(np.ascontiguousarray(bcr))
        in_maps.append({**common, "wt": slab, "bias_m": bc})
    return in_maps


def _build_fast(nc):
    """Cache a jitted executor for this nc so repeat kernel() calls skip
    retracing/recompiling (mirrors bass2jax.run_bass_via_pjrt)."""
    import jax
    from concourse import bass2jax
    from concourse.bass2jax import _bass_exec_p, partition_id_tensor
    from jax.experimental.shard_map import shard_map
    from jax.sharding import Mesh, NamedSharding, PartitionSpec

    bass2jax.install_neuronx_cc_hook()
    partition_name = nc.partition_id_tensor.name if nc.partition_id_tensor else None
    in_names, out_names, out_avals, zero_outs = [], [], [], []
    for alloc in nc.m.functions[0].allocations:
        if not isinstance(alloc, mybir.MemoryLocationSet):
            continue
        name = alloc.memorylocations[0].name
        if alloc.kind == "ExternalInput":
            if name != partition_name:
                in_names.append(name)
        elif alloc.kind == "ExternalOutput":
            out_names.append(name)
            shape = tuple(alloc.tensor_shape)
            dtype = mybir.dt.np(alloc.dtype)
            out_avals.append(jax.core.ShapedArray(shape, dtype))
            zero_outs.append(np.zeros(shape, dtype))
    n_params = len(in_names)
    all_in = list(in_names) + list(out_names)
    if partition_name is not None:
        all_in.append(partition_name)

    def _body(*args):
        ops = list(args)
        if partition_name is not None:
            ops.append(partition_id_tensor())
        return tuple(
            _bass_exec_p.bind(
                *ops,
                out_avals=tuple(out_avals),
                in_names=tuple(all_in),
                out_names=tuple(out_names),
                lowering_input_output_aliases=(),
                sim_require_finite=True,
                sim_require_nnan=True,
                nc=nc,
            )
        )

    devices = jax.devices()[:NCORES]
    mesh = Mesh(np.asarray(devices), ("core",))
    spec = PartitionSpec("core")
    fn = jax.jit(
        shard_map(
            _body, mesh=mesh,
            in_specs=(spec,) * (n_params + len(out_names)),
            out_specs=(spec,) * len(out_names), check_rep=False,
        ),
        keep_unused=True,
    )
    _fast[id(nc)] = (fn, in_names, out_names, out_avals, zero_outs, mesh, spec)


def _run_cached(nc, in_maps):
    import jax

    fn, in_names, out_names, out_avals, zero_outs, mesh, spec = _fast[id(nc)]
    concat_in = [
        np.concatenate([np.asarray(in_maps[c][nm]) for c in range(NCORES)], axis=0)
        for nm in in_names
    ]
    concat_zero = [
        np.zeros((NCORES * z.shape[0], *z.shape[1:]), z.dtype) for z in zero_outs
    ]
    outs = fn(*concat_in, *concat_zero)
    return [
        {
            nm: np.asarray(outs[i]).reshape(NCORES, *out_avals[i].shape)[c]
            for i, nm in enumerate(out_names)
        }
        for c in range(NCORES)
    ]


def kernel(last_hidden_state, param_w, param_b, points):
    global _last_results
    from concourse.bass_utils import run_bass_kernel_spmd

    lhs = np.asarray(last_hidden_state, dtype=np.float32)
    W = np.ascontiguousarray(np.asarray(param_w, dtype=np.float32))
    bias_vec = np.asarray(param_b, dtype=np.float32)
    pts = np.asarray(points)

    ht = np.ascontiguousarray(lhs[:, -1, :].T)  # (D, B)
    in_maps = _prep_core_inputs(W, bias_vec, pts, ht)

    # the bias-first matmul doubles as the psum wait-absorber, which
    # pipelines slightly better than the no-bias variant even for zero bias
    nc = _get_nc(use_bias=True)
    if id(nc) in _fast:
        results = _run_cached(nc, in_maps)
    else:
        res = run_bass_kernel_spmd(nc, in_maps, core_ids=list(range(NCORES)))
        _last_results = res
        results = res.results
        _build_fast(nc)

    s = np.zeros((B, NCHUNK), np.float64)
    for r in results:
        s += r["sg_out"][:, :NCHUNK].astype(np.float64)
    g = results[0]["sg_out"][:, NCHUNK:].astype(np.float64)
    s0, s1 = s[:, :R], s[:, R:]
    g0, g1 = g[:, :R], g[:, R:]
    norm_const = (s0 * s1).sum(axis=1)
    p_eval = (g0 * g1).sum(axis=1)
    return p_eval.astype(np.float32), norm_const.astype(np.float32)

